# revision 12
# baseline (speedup 1.0000x reference)
"""MoE LTS-memory kernel for Trainium2, token-parallel over 8 NeuronCores.

Strategy: flatten (B,S) -> 8192 tokens, shard 1024 tokens/core. All weights
(router, in/out proj, memory banks in both [c,m] and [m,c] layouts) are
replicated; activations are shipped pre-transposed ([d, t] layouts) so every
matmul contraction dim lands on SBUF partitions with no on-device transposes
of the big activations. Matmuls run as float32r (TF32-class, 1 cyc/row).

Per core (T=1024 tokens, chunks of 128):
  mem_qT[m,t] = in_wT.T @ qT (+in_b)            [512, 1024]
  logitsT[e,t] = router_wT.T @ hsT -> transpose -> routing math on [t,8]
  per expert e:
    attn[c,t] = memT_e.T @ mem_qT               (PSUM, [c,1024])
    exp = Exp(attn/sqrt(512))                   (ACT, -> f32r SBUF)
    U[t,m] = exp.T @ mem_nat_e ; s[t] = exp.T @ ones   (PSUM)
    comb[t,m] += (gate_e[t]/s[t]) * U[t,m]      (DVE scalar_tensor_tensor)
  out[t,d] = combT.T @ out_wT + out_b           [1024, 2048]
Load-balancing loss partial sums (top1 one-hot counts, softmax prob sums)
are reduced on-device to [1,8] each and finished on host.
"""
import sys, os, math
sys.path.insert(0, '/opt/trn_rl_repo')
import numpy as np

B, S = 4, 2048
D_MODEL, D_MEM = 2048, 512
E, TOPK, CAP = 8, 2, 1024
NCORES = 8
T = (B * S) // NCORES        # 1024 tokens per core
TC = T // 128                # 8 token chunks
DC = D_MODEL // 128          # 16
MC = D_MEM // 128            # 4
CC = CAP // 128              # 8
INV_SQRT_DMEM = 1.0 / math.sqrt(D_MEM)

_CACHE = {}


def _build():
    import concourse.bass as bass
    import concourse.mybir as mybir
    import concourse.tile as tile
    from concourse import bacc
    from concourse.masks import make_identity

    f32 = mybir.dt.float32
    f32r = mybir.dt.float32r
    AX = mybir.AxisListType.X
    OP = mybir.AluOpType
    AF = mybir.ActivationFunctionType

    nc = bacc.Bacc("TRN2", target_bir_lowering=False, debug=False,
                   num_devices=NCORES)

    with tile.TileContext(nc) as tc:
        with tc.tile_pool(name="dram", bufs=1, space="DRAM") as dram:
            d_qT = dram.tile([D_MODEL, T], f32r, kind="ExternalInput", name="qT")
            d_hsT = dram.tile([D_MODEL, T], f32, kind="ExternalInput", name="hsT")
            d_iwT = dram.tile([D_MODEL, D_MEM], f32r, kind="ExternalInput", name="iwT")
            d_rwT = dram.tile([D_MODEL, E], f32, kind="ExternalInput", name="rwT")
            d_ibT = dram.tile([128, MC], f32, kind="ExternalInput", name="ibT")
            d_memT = dram.tile([E, D_MEM, CAP], f32r, kind="ExternalInput", name="memT")
            d_memN = dram.tile([E, CAP, D_MEM], f32r, kind="ExternalInput", name="memN")
            d_owT = dram.tile([D_MEM, D_MODEL], f32r, kind="ExternalInput", name="owT")
            d_obB = dram.tile([128, D_MODEL], f32, kind="ExternalInput", name="obB")
            d_out = dram.tile([T, D_MODEL], f32, kind="ExternalOutput", name="out")
            d_st = dram.tile([1, 2 * E], f32, kind="ExternalOutput", name="st")

            with tc.tile_pool(name="const", bufs=1) as const, \
                 tc.tile_pool(name="qpool", bufs=3) as qpool, \
                 tc.tile_pool(name="hpool", bufs=3) as hpool, \
                 tc.tile_pool(name="iwpool", bufs=3) as iwpool, \
                 tc.tile_pool(name="mqpool", bufs=1) as mqpool, \
                 tc.tile_pool(name="mtpool", bufs=1) as mtpool, \
                 tc.tile_pool(name="mnpool", bufs=1) as mnpool, \
                 tc.tile_pool(name="expool", bufs=1) as expool, \
                 tc.tile_pool(name="rpool", bufs=2) as rpool, \
                 tc.tile_pool(name="cpool", bufs=1) as cpool, \
                 tc.tile_pool(name="ctpool", bufs=2) as ctpool, \
                 tc.tile_pool(name="opool", bufs=2) as opool, \
                 tc.tile_pool(name="psA", bufs=2, space="PSUM") as psA, \
                 tc.tile_pool(name="psB", bufs=2, space="PSUM") as psB, \
                 tc.tile_pool(name="psS", bufs=2, space="PSUM") as psS:

                # ---- constants ----
                ident = const.tile([128, 128], f32)
                make_identity(nc, ident[:])
                ones_f = const.tile([128, 1], f32)
                nc.vector.memset(ones_f[:], 1.0)
                ones_r = const.tile([128, 2], f32r)
                nc.scalar.copy(ones_r[:, 0:1], ones_f[:])
                nc.scalar.copy(ones_r[:, 1:2], ones_f[:])
                rw_sb = const.tile([128, DC, E], f32)
                nc.sync.dma_start(rw_sb[:], d_rwT[:].rearrange("(dc p) e -> p dc e", p=128))
                ib_sb = const.tile([128, MC], f32)
                nc.sync.dma_start(ib_sb[:], d_ibT[:])
                ob_sb = const.tile([128, D_MODEL], f32)
                nc.sync.dma_start(ob_sb[:], d_obB[:])
                owT_sb = const.tile([128, MC, D_MODEL], f32r)
                nc.sync.dma_start(owT_sb[:], d_owT[:].rearrange("(mc p) d -> p mc d", p=128))

                # ---- mm1: mem_qT[m, t] (f32r) ----
                mq_sb = mqpool.tile([128, MC, T], f32r)
                qT_r = d_qT[:].rearrange("(dc p) t -> dc p t", p=128)
                iw_r = d_iwT[:].rearrange("(dc p) m -> dc p m", p=128)
                for pair in range(2):
                    pms = [psA.tile([128, T], f32, tag="pa", name=f"pm1_{pair}_{i}")
                           for i in range(2)]
                    for dc in range(DC):
                        qt = qpool.tile([128, T], f32r, name="qt")
                        nc.sync.dma_start(qt[:], qT_r[dc])
                        iw = iwpool.tile([128, D_MEM], f32r, name="iw")
                        nc.sync.dma_start(iw[:], iw_r[dc])
                        for i in range(2):
                            mc = pair * 2 + i
                            for h in range(2):
                                nc.tensor.matmul(
                                    pms[i][:, h * 512:(h + 1) * 512],
                                    iw[:, mc * 128:(mc + 1) * 128],
                                    qt[:, h * 512:(h + 1) * 512],
                                    start=(dc == 0), stop=(dc == DC - 1))
                    for i in range(2):
                        mc = pair * 2 + i
                        nc.scalar.activation(mq_sb[:, mc, :], pms[i][:],
                                             AF.Identity,
                                             bias=ib_sb[:, mc:mc + 1], scale=1.0)

                # ---- router: logitsT[e, t] then routing math ----
                pr = psA.tile([8, T], f32, tag="pa", name="pr")
                hsT_r = d_hsT[:].rearrange("(dc p) t -> dc p t", p=128)
                for dc in range(DC):
                    ht = hpool.tile([128, T], f32, name="ht")
                    nc.sync.dma_start(ht[:], hsT_r[dc])
                    for h in range(2):
                        nc.tensor.matmul(pr[:, h * 512:(h + 1) * 512],
                                         rw_sb[:, dc, :],
                                         ht[:, h * 512:(h + 1) * 512],
                                         start=(dc == 0), stop=(dc == DC - 1))
                lg_sb = rpool.tile([8, T], f32, name="lg_sb", bufs=1)
                nc.scalar.copy(lg_sb[:], pr[:])

                # transpose logits chunks -> [128t, e], then per-chunk routing
                ws_sb = rpool.tile([128, TC, E], f32, name="ws_sb", bufs=1)
                w_sb = rpool.tile([128, TC, E], f32, name="w_sb", bufs=1)
                m1_sb = rpool.tile([128, TC, E], f32, name="m1_sb", bufs=1)
                for t in range(TC):
                    plt = psB.tile([128, 8], f32, tag="pb", name="plt")
                    nc.tensor.matmul(plt[:], lg_sb[:, t * 128:(t + 1) * 128],
                                     ident[0:8, 0:8], is_transpose=True,
                                     start=True, stop=True)
                    lgt = rpool.tile([128, E], f32, name="lgt")
                    nc.vector.tensor_copy(lgt[:], plt[:])
                    mx = rpool.tile([128, 1], f32, name="mx")
                    nc.vector.reduce_max(mx[:], lgt[:], axis=AX, negate=True)
                    s8 = rpool.tile([128, 1], f32, name="s8")
                    nc.scalar.activation(w_sb[:, t, :], lgt[:], AF.Exp,
                                         bias=mx[:], scale=1.0, accum_out=s8[:])
                    rs8 = rpool.tile([128, 1], f32, name="rs8")
                    nc.vector.reciprocal(rs8[:], s8[:])
                    nc.vector.tensor_scalar_mul(w_sb[:, t, :], w_sb[:, t, :], rs8[:])
                    t1 = rpool.tile([128, 1], f32, name="t1")
                    nc.vector.reduce_max(t1[:], w_sb[:, t, :], axis=AX)
                    nc.vector.tensor_scalar(m1_sb[:, t, :], w_sb[:, t, :], t1[:],
                                            None, op0=OP.is_equal)
                    w2 = rpool.tile([128, E], f32, name="w2")
                    # w2 = w * (1 - m1)  == w - m1*w
                    nc.vector.tensor_tensor(w2[:], m1_sb[:, t, :], w_sb[:, t, :],
                                            op=OP.mult)
                    nc.vector.tensor_tensor(w2[:], w_sb[:, t, :], w2[:],
                                            op=OP.subtract)
                    t2 = rpool.tile([128, 1], f32, name="t2")
                    nc.vector.reduce_max(t2[:], w2[:], axis=AX)
                    m2 = rpool.tile([128, E], f32, name="m2")
                    nc.vector.tensor_scalar(m2[:], w2[:], t2[:], None,
                                            op0=OP.is_equal)
                    den = rpool.tile([128, 1], f32, name="den")
                    nc.vector.tensor_tensor(den[:], t1[:], t2[:], op=OP.add)
                    nc.vector.tensor_scalar_add(den[:], den[:], 1e-8)
                    rden = rpool.tile([128, 1], f32, name="rden")
                    nc.vector.reciprocal(rden[:], den[:])
                    g1 = rpool.tile([128, E], f32, name="g1")
                    nc.vector.tensor_scalar(g1[:], m1_sb[:, t, :], t1[:], rden[:],
                                            op0=OP.mult, op1=OP.mult)
                    g2 = rpool.tile([128, E], f32, name="g2")
                    nc.vector.tensor_scalar(g2[:], m2[:], t2[:], rden[:],
                                            op0=OP.mult, op1=OP.mult)
                    nc.vector.tensor_tensor(ws_sb[:, t, :], g1[:], g2[:], op=OP.add)

                # stats: dispatch counts (sum m1) and prob sums (sum w), f32 matmul
                pst_d = psB.tile([1, E], f32, tag="pb", name="pst_d")
                for t in range(TC):
                    nc.tensor.matmul(pst_d[:], ones_f[:], m1_sb[:, t, :],
                                     start=(t == 0), stop=(t == TC - 1))
                pst_p = psB.tile([1, E], f32, tag="pb", name="pst_p")
                for t in range(TC):
                    nc.tensor.matmul(pst_p[:], ones_f[:], w_sb[:, t, :],
                                     start=(t == 0), stop=(t == TC - 1))
                st_sb = rpool.tile([1, 2 * E], f32, name="st_sb")
                nc.vector.tensor_copy(st_sb[:, 0:E], pst_d[:])
                nc.vector.tensor_copy(st_sb[:, E:2 * E], pst_p[:])
                nc.sync.dma_start(d_st[:], st_sb[:])

                # ---- expert loop ----
                comb_sb = cpool.tile([128, TC, D_MEM], f32)
                rs_sb = rpool.tile([128, TC, E], f32, name="rs_sb", bufs=1)
                memT_r = d_memT[:].rearrange("e (mc p) c -> e mc p c", p=128)
                memN_r = d_memN[:].rearrange("e (cc p) m -> e cc p m", p=128)
                for e in range(E):
                    mt = mtpool.tile([128, MC, CAP], f32r, name="mt")
                    nc.sync.dma_start(mt[:], memT_r[e].rearrange("mc p c -> p mc c"))
                    mn = mnpool.tile([128, CC, D_MEM], f32r, name="mn")
                    nc.sync.dma_start(mn[:], memN_r[e].rearrange("cc p m -> p cc m"))
                    ex = expool.tile([128, CC, T], f32r, name="ex")
                    for cc in range(CC):
                        pa = psA.tile([128, T], f32, tag="pa", name="pa2")
                        for mc in range(MC):
                            for h in range(2):
                                nc.tensor.matmul(
                                    pa[:, h * 512:(h + 1) * 512],
                                    mt[:, mc, cc * 128:(cc + 1) * 128],
                                    mq_sb[:, mc, h * 512:(h + 1) * 512],
                                    start=(mc == 0), stop=(mc == MC - 1))
                        nc.scalar.activation(ex[:, cc, :], pa[:], AF.Exp,
                                             bias=0.0, scale=INV_SQRT_DMEM)
                    for t in range(TC):
                        pu = psB.tile([128, D_MEM], f32, tag="pb", name="pu")
                        psc = psS.tile([128, 2], f32, name="psc")
                        for cc in range(CC):
                            lhs = ex[:, cc, t * 128:(t + 1) * 128]
                            nc.tensor.matmul(pu[:], lhs, mn[:, cc, :],
                                             start=(cc == 0), stop=(cc == CC - 1))
                            nc.tensor.matmul(psc[:], lhs, ones_r[:],
                                             start=(cc == 0), stop=(cc == CC - 1))
                        nc.vector.reciprocal(rs_sb[:, t, e:e + 1], psc[:, 0:1])
                        wsf = rpool.tile([128, 1], f32, name="wsf")
                        nc.vector.tensor_tensor(wsf[:], ws_sb[:, t, e:e + 1],
                                                rs_sb[:, t, e:e + 1], op=OP.mult)
                        if e == 0:
                            nc.vector.tensor_scalar(comb_sb[:, t, :], pu[:],
                                                    wsf[:], None, op0=OP.mult)
                        else:
                            nc.vector.scalar_tensor_tensor(
                                comb_sb[:, t, :], pu[:], wsf[:],
                                comb_sb[:, t, :], op0=OP.mult, op1=OP.add)

                # ---- mm4: out[t, d] = combT.T @ out_wT + out_b ----
                out_r = d_out[:].rearrange("(tc p) d -> tc p d", p=128)
                for t in range(TC):
                    ct = ctpool.tile([128, MC, 128], f32r, name="ct")
                    for mc in range(MC):
                        ptr = psB.tile([128, 128], f32, tag="pb", name="ptr")
                        nc.tensor.matmul(ptr[:],
                                         comb_sb[:, t, mc * 128:(mc + 1) * 128],
                                         ident[:], is_transpose=True,
                                         start=True, stop=True)
                        nc.scalar.copy(ct[:, mc, :], ptr[:])
                    ot = opool.tile([128, D_MODEL], f32, name="ot")
                    for dt in range(4):
                        po = psB.tile([128, 512], f32, tag="pb", name="po")
                        for mc in range(MC):
                            nc.tensor.matmul(po[:], ct[:, mc, :],
                                             owT_sb[:, mc, dt * 512:(dt + 1) * 512],
                                             start=(mc == 0), stop=(mc == MC - 1))
                        nc.vector.tensor_tensor(ot[:, dt * 512:(dt + 1) * 512],
                                                po[:],
                                                ob_sb[:, dt * 512:(dt + 1) * 512],
                                                op=OP.add)
                    nc.sync.dma_start(out_r[t], ot[:])

    nc.compile()
    names = dict(qT=d_qT.name, hsT=d_hsT.name, iwT=d_iwT.name, rwT=d_rwT.name,
                 ibT=d_ibT.name, memT=d_memT.name, memN=d_memN.name,
                 owT=d_owT.name, obB=d_obB.name, out=d_out.name, st=d_st.name)
    return nc, names


def kernel(hidden_states, query, router_w, memories, in_w, in_b, out_w, out_b,
           _trace=False):
    from concourse.bass_utils import run_bass_kernel_spmd

    if "nc" not in _CACHE:
        _CACHE["nc"], _CACHE["names"] = _build()
    nc, names = _CACHE["nc"], _CACHE["names"]

    hs = np.ascontiguousarray(np.asarray(hidden_states, np.float32).reshape(B * S, D_MODEL).T)
    qT = np.ascontiguousarray(np.asarray(query, np.float32).reshape(B * S, D_MODEL).T)
    iwT = np.ascontiguousarray(np.asarray(in_w, np.float32).T)
    rwT = np.ascontiguousarray(np.asarray(router_w, np.float32).T)
    ibT = np.ascontiguousarray(np.asarray(in_b, np.float32).reshape(MC, 128).T)
    memN = np.ascontiguousarray(np.asarray(memories, np.float32))
    memT = np.ascontiguousarray(memN.transpose(0, 2, 1))
    owT = np.ascontiguousarray(np.asarray(out_w, np.float32).T)
    obB = np.ascontiguousarray(np.broadcast_to(np.asarray(out_b, np.float32),
                                               (128, D_MODEL)))
    in_maps = []
    for c in range(NCORES):
        sl = slice(c * T, (c + 1) * T)
        in_maps.append({
            names["qT"]: np.ascontiguousarray(qT[:, sl]),
            names["hsT"]: np.ascontiguousarray(hs[:, sl]),
            names["iwT"]: iwT, names["rwT"]: rwT, names["ibT"]: ibT,
            names["memT"]: memT, names["memN"]: memN,
            names["owT"]: owT, names["obB"]: obB,
        })

    res = run_bass_kernel_spmd(nc, in_maps, core_ids=list(range(NCORES)),
                               trace=_trace)
    out = np.concatenate([res.results[c][names["out"]] for c in range(NCORES)],
                         axis=0).reshape(B, S, D_MODEL)
    st = np.stack([res.results[c][names["st"]][0] for c in range(NCORES)])
    dispatch = st[:, 0:E].sum(0) / (B * S)
    prob = st[:, E:2 * E].sum(0) / (B * S)
    loss = np.float32(E * np.sum(dispatch * prob))
    if _trace:
        kernel._last_exec_ns = res.exec_time_ns
    return out, loss
